# revision 20
# baseline (speedup 1.0000x reference)
"""Causal self-attention nn module (B=4, T=2048, E=1024, H=16, HS=64) on 8
TRN2 cores — faithful to the reference's raw .view() reshape [b,t,h,hs] ->
[h,b,t,hs].

That reshape makes the attention run over 64 independent "sequences": each
sequence is one 128-timestep block of one batch, with its 16 heads
interleaved into 2048 positions (t2 = tau*16 + h).  Sequence (b, s') covers
x[b, 128*s' : 128*(s'+1), :], and its attention output lands back in rows
[128*s', 128*(s'+1)) of att_cat[b] — so sharding by sequence blocks needs no
cross-core reduction at all.

Sharding: core c handles batch b = c//2, rows t in [1024*(c%2), +1024) — 8
sequences.  Each core computes full rows of the output; host concatenates
and adds proj_b.

Dtypes: phase A/C matmuls and V/P tiles are bf16 (PSUM accumulation fp32).
Qseq/Kseq stay fp32r — their PSUM->SBUF scatter writes are 2-byte-strided
in bf16 and run 2.3x slower on DVE, while fp32r matmul is the same
1 cycle/row as bf16 at N>=256.

Schedule (the real optimization): the PE re-throttles (HAM K=4/8 ==
1.2 GHz) whenever it micro-stalls on the softmax chain, so the emission
order is built to keep the PE stream dependency-free:
  - All input DMAs are issued upfront as single whole-tensor transfers
    (the v DRAM round-trip gather is 1 DMA per sequence).
  - Phase A emits only the q/k projections for sequence pairs 0-1 before
    attention starts; the pairs 2-3 projections are interleaved into the
    attention stream of pairs 0-1 as PE filler (their PSUM->Qseq scatter
    copies split across DVE and the otherwise-idle ACT).
  - Attention groups keep score matmuls one k-tile ahead of the P.T @ V
    accumulation; scores for both packed sequences land in one 2-bank
    PSUM tile so exps are merged-pair ACT instructions.
  - Normalization (ones-block denominators -> ln -> exp(-x) -> one
    in-place DVE multiply) is batched per pair and deferred one pair;
    the projection (phase C) for pair sp is interleaved two pairs later,
    giving the PE independent work at every group boundary.
"""

import numpy as np
from contextlib import ExitStack

import concourse.bass as bass
import concourse.mybir as mybir
import concourse.tile as tile
from concourse import bacc
from concourse.bass_utils import run_bass_kernel_spmd

F32 = mybir.dt.float32
F32R = mybir.dt.float32r
BF16 = mybir.dt.bfloat16
AF = mybir.ActivationFunctionType

B, T, E, H, HS = 4, 2048, 1024, 16, 64
NCORES = 8
TCORE = T * B // NCORES      # 1024 rows per core
DH = H * HS                  # 1024
NEG = -1.0e9
SCALE = HS ** -0.5

# band tile geometry: for diagonal-band tile jj, scores only needed in
# columns [128*jj, 512); matmul N kept >=256 for full-rate fp32r.
BAND_C0 = [0, 128, 256, 256]     # first column the st matmul writes
BAND_A0 = [0, 128, 256, 256]     # first column the @v matmul writes


def build_nc(t_core=TCORE, e=E, e_out=E):
    assert t_core % 512 == 0
    nseq = t_core // 128     # sequences (= tau tiles)
    nsp = nseq // 2          # sequence pairs
    ne = e // 128
    nhp = H // 2             # 8 head pairs
    ntk = 16                 # t2 tiles per sequence (2048/128)
    niq = 4                  # t2 query blocks (2048/512)
    neo = e_out // 512

    nc = bacc.Bacc("TRN2", debug=False, num_devices=1)

    xT_d = nc.dram_tensor("xT", [e, t_core], BF16, kind="ExternalInput")
    wq_d = nc.dram_tensor("wq", [e, DH], BF16, kind="ExternalInput")
    wk_d = nc.dram_tensor("wk", [e, DH], BF16, kind="ExternalInput")
    wv_d = nc.dram_tensor("wv", [e, DH], BF16, kind="ExternalInput")
    pw_d = nc.dram_tensor("pwT", [DH, e_out], BF16, kind="ExternalInput")
    tri_d = nc.dram_tensor("tri", [128, 128], F32, kind="ExternalInput")
    y_d = nc.dram_tensor("y", [t_core, e_out], F32, kind="ExternalOutput")

    with tile.TileContext(nc) as tc, ExitStack() as ctx:
        p_keep = ctx.enter_context(tc.tile_pool(name="keep", bufs=1))
        Qseq = p_keep.tile([128, nsp, 2048], F32R, tag="Qseq")
        Kseq = p_keep.tile([128, nsp, 2048], F32R, tag="Kseq")
        tri_sb = p_keep.tile([128, 128], F32, tag="tri")
        nc.sync.dma_start(out=tri_sb, in_=tri_d.ap())

        # per (pair, tk) stationary [128 t2, 192]: cols 0:64 = V of seq A,
        # 64:128 = ones (replicates the softmax denominator across 64 PSUM
        # partitions in the @v matmul), 128:192 = V of seq B
        p_vsb = ctx.enter_context(tc.tile_pool(name="vsb", bufs=1))
        v_sb = p_vsb.tile([128, nsp, ntk, 192], BF16, tag="v")
        nc.vector.memset(v_sb, 1.0)

        p_drm = ctx.enter_context(tc.tile_pool(name="drm", bufs=1, space="DRAM"))
        vscr = p_drm.tile([t_core, DH], BF16, tag="vscr")

        # pools are stack-scoped per memory space: wv sits on top of the
        # SBUF stack (closed right after the v matmuls), Aps on top of the
        # PSUM stack (closed when the projection pool opens)
        p_x = ctx.enter_context(tc.tile_pool(name="px", bufs=1))
        p_w = ctx.enter_context(tc.tile_pool(name="pw", bufs=1))
        p_stg = ctx.enter_context(tc.tile_pool(name="vstg", bufs=2))
        p_st = ctx.enter_context(tc.tile_pool(name="st_ps", bufs=2, space="PSUM"))
        p_av = ctx.enter_context(tc.tile_pool(name="av_ps", bufs=3, space="PSUM"))
        wv_ctx = ExitStack()
        p_wv = wv_ctx.enter_context(tc.tile_pool(name="pwv", bufs=1))
        aps_ctx = ExitStack()
        p_Aps = aps_ctx.enter_context(
            tc.tile_pool(name="Aps", bufs=1, space="PSUM")
        )

        # ---- input DMAs, one per tensor (x split for earlier start) ----
        xT = p_x.tile([128, ne, t_core], BF16, tag="xT")
        x_r = xT_d.ap().rearrange("(a p) t -> p a t", p=128)
        w_sb = {}

        def load_w(name, dram, pool):
            w = pool.tile([128, ne, DH], BF16, tag=name)
            nc.sync.dma_start(
                out=w, in_=dram.ap().rearrange("(a p) d -> p a d", p=128)
            )
            w_sb[name] = w

        # x and wv first (the v chains need them); wq/wk can land later
        nc.sync.dma_start(out=xT[:, 0 : ne // 2, :], in_=x_r[:, 0 : ne // 2, :])
        load_w("wv", wv_d, p_wv)
        nc.sync.dma_start(out=xT[:, ne // 2 :, :], in_=x_r[:, ne // 2 :, :])
        load_w("wq", wq_d, p_w)
        load_w("wk", wk_d, p_w)

        # ---------------- phase A (v + q/k for pairs 0-1) ----------------
        # v -> DRAM scratch -> gathered V tiles (t2 on partitions); the
        # round-trip latency hides under the q/k matmuls.  The DVE copy
        # also casts fp32 PSUM -> bf16 (DMA cannot convert dtypes).
        for tt in range(nseq):
            stg = p_stg.tile([128, DH], BF16, tag="stg")
            for c2 in range(DH // 512):
                ps = p_Aps.tile([128, 512], F32, tag="Aps", name="Aps")
                for ei in range(ne):
                    nc.tensor.matmul(
                        ps,
                        xT[:, ei, bass.ts(tt, 128)],
                        w_sb["wv"][:, ei, bass.ts(c2, 512)],
                        start=(ei == 0),
                        stop=(ei == ne - 1),
                    )
                nc.vector.tensor_copy(stg[:, bass.ts(c2, 512)], ps)
            nc.sync.dma_start(out=vscr[bass.ts(tt, 128), :], in_=stg)
        wv_ctx.close()
        for j in range(nseq):
            src = vscr[j * 128 : (j + 1) * 128, :].rearrange(
                "(b a) (h c) -> (a h) b c", a=8, c=64
            )
            c0 = 128 * (j % 2)
            nc.sync.dma_start(
                out=v_sb[:, j // 2, :, c0 : c0 + 64], in_=src
            )

        def emit_qk_chunk(name, hp, c, use_act):
            # one PSUM chain of the q/k projection + its scatter into the
            # interleaved-transposed layout: seq j = 4c+j4, head h = 2hp+hh,
            # col t2 = tau*16 + h
            dst = Qseq if name == "wq" else Kseq
            w = w_sb[name]
            ps = p_Aps.tile([128, 512], F32, tag="Aps", name="Aps")
            for ei in range(ne):
                nc.tensor.matmul(
                    ps,
                    w[:, ei, hp * 128 : hp * 128 + 128],
                    xT[:, ei, bass.ts(c, 512)],
                    start=(ei == 0),
                    stop=(ei == ne - 1),
                )
            for hh in range(2):
                h = 2 * hp + hh
                pv = ps[64 * hh : 64 * hh + 64, :].rearrange(
                    "p (j4 tau) -> p j4 tau", j4=4
                )
                for par in range(2):
                    dv = dst[64 * par : 64 * par + 64, :, :].rearrange(
                        "p sp (tau hx) -> p sp tau hx", hx=16
                    )
                    if use_act and hh == 1:
                        nc.scalar.activation(
                            dv[:, 2 * c : 2 * c + 2, :, h],
                            pv[:, par::2, :],
                            AF.Copy,
                        )
                    else:
                        nc.vector.tensor_copy(
                            dv[:, 2 * c : 2 * c + 2, :, h],
                            pv[:, par::2, :],
                        )

        for name in ("wq", "wk"):
            for hp in range(nhp):
                emit_qk_chunk(name, hp, 0, use_act=True)

        # q/k for pairs 2-3: emitted as filler inside the attention stream
        filler = [
            (name, hp) for name in ("wq", "wk") for hp in range(nhp)
        ]

        # ---------------- phase B + interleaved A-tail and C ----------------
        p_big = ctx.enter_context(tc.tile_pool(name="big", bufs=1))
        attC = p_big.tile([128, 8, t_core], BF16, tag="attC")
        pwT = p_big.tile([128, 8, e_out], BF16, tag="pwT")
        nc.sync.dma_start(
            out=pwT, in_=pw_d.ap().rearrange("(g p) E -> p g E", p=128)
        )

        p_at = ctx.enter_context(tc.tile_pool(name="attn", bufs=3))
        p_nrm = ctx.enter_context(tc.tile_pool(name="nrm", bufs=2))
        p_dns = ctx.enter_context(tc.tile_pool(name="dns", bufs=3))
        p_dnl = ctx.enter_context(tc.tile_pool(name="dnl", bufs=1))
        p_out = ctx.enter_context(tc.tile_pool(name="out", bufs=2))
        p_Cps = None  # opened after the phase-A PSUM pool closes

        dens_t = {}

        def emit_norm(sp):
            # 1/denominator = exp(-ln d) batched for the whole pair, then
            # one fused in-place multiply over attC's two column blocks
            dens = dens_t.pop(sp)
            dnl = p_dnl.tile([128, 8, 256], F32, tag="dnl", name="dnl")
            nc.scalar.activation(dnl, dens, AF.Ln)
            nc.scalar.activation(dens, dnl, AF.Exp, scale=-1.0)
            sl_ = attC[:, :, 2 * sp * 128 : 2 * sp * 128 + 256]
            nc.vector.tensor_mul(sl_, sl_, dens)

        def emit_proj(tt):
            for eh in range(neo):
                ps = p_Cps.tile([128, 512], F32, tag="Cps", name="Cps")
                for g in range(8):
                    nc.tensor.matmul(
                        ps,
                        attC[:, g, bass.ts(tt, 128)],
                        pwT[:, g, bass.ts(eh, 512)],
                        start=(g == 0),
                        stop=(g == 7),
                    )
                y_sb = p_out.tile([128, 512], F32, tag="y")
                nc.vector.tensor_copy(y_sb, ps)
                nc.sync.dma_start(
                    out=y_d.ap()[bass.ts(tt, 128), bass.ts(eh, 512)],
                    in_=y_sb,
                )

        def emit_group(sp, iq):
            n_tk = 4 * (iq + 1)
            av = {}
            for hh in range(2):
                av[hh] = p_av.tile([128, 512], F32, tag="av", name="av")
            sts = {}

            def emit_st(tk):
                jj = tk - 4 * iq
                c0 = BAND_C0[jj] if jj >= 0 else 0
                t_ = p_st.tile([128, 2, 512], F32, tag="st", name="st")
                for hh in range(2):
                    sl = slice(64 * hh, 64 * hh + 64)
                    nc.tensor.matmul(
                        t_[:, hh, c0:512],
                        Kseq[sl, sp, bass.ts(tk, 128)],
                        Qseq[sl, sp, iq * 512 + c0 : (iq + 1) * 512],
                        start=True,
                        stop=True,
                    )
                sts[tk] = t_

            emit_st(0)
            for tk in range(n_tk):
                if tk + 1 < n_tk:
                    emit_st(tk + 1)
                jj = tk - 4 * iq
                st = sts.pop(tk)
                pt = p_at.tile([128, 2, 512], BF16, tag="pt", name="pt")
                if jj < 0:
                    nc.scalar.activation(pt, st, AF.Exp, scale=SCALE)
                    a0 = 0
                else:
                    d0 = 128 * jj
                    if d0 > 0:
                        nc.vector.memset(pt[:, :, 0:d0], 0.0)
                    sm = p_nrm.tile([128, 2, 128], F32, tag="sm", name="sm")
                    for hh in range(2):
                        nc.vector.tensor_add(
                            sm[:, hh, :], st[:, hh, d0 : d0 + 128], tri_sb
                        )
                    nc.scalar.activation(
                        pt[:, :, d0 : d0 + 128], sm, AF.Exp, scale=SCALE
                    )
                    if d0 + 128 < 512:
                        nc.scalar.activation(
                            pt[:, :, d0 + 128 : 512],
                            st[:, :, d0 + 128 : 512],
                            AF.Exp,
                            scale=SCALE,
                        )
                    a0 = BAND_A0[jj]
                for hh in range(2):
                    nc.tensor.matmul(
                        av[hh][:, a0:512],
                        v_sb[:, sp, tk, 64 * hh : 64 * hh + 128],
                        pt[:, hh, a0:512],
                        start=(tk == 0),
                        stop=(tk == n_tk - 1),
                        skip_group_check=True,
                    )
            # av rows: hh=0 -> out 0:64, denominator 64:128; hh=1 mirrored.
            # Copy out rows unnormalized into attC.T layout; stash the
            # denominators in matching layout for the per-pair batch.
            dens = dens_t[sp]
            for hh in range(2):
                j = 2 * sp + hh
                o0 = 64 * hh
                d0 = 64 - o0
                avv = av[hh][o0 : o0 + 64, :].rearrange(
                    "p (tau g r) -> p r g tau", tau=32, g=8, r=2
                )
                dvv = av[hh][d0 : d0 + 64, :].rearrange(
                    "p (tau g r) -> p r g tau", tau=32, g=8, r=2
                )
                for r in range(2):
                    nc.vector.tensor_copy(
                        attC[
                            64 * r : 64 * r + 64,
                            :,
                            j * 128 + iq * 32 : j * 128 + iq * 32 + 32,
                        ],
                        avv[:, r, :, :],
                    )
                    nc.vector.tensor_copy(
                        dens[
                            64 * r : 64 * r + 64,
                            :,
                            hh * 128 + iq * 32 : hh * 128 + iq * 32 + 32,
                        ],
                        dvv[:, r, :, :],
                    )

        # interleave schedule: pair 0/1 groups carry the pairs-2-3 q/k
        # filler; pair 2/3 groups carry normalization + projection filler
        for sp in range(nsp):
            dens_t[sp] = p_dns.tile(
                [128, 8, 256], BF16, tag="dens", name="dens"
            )
            for iq in range(niq):
                emit_group(sp, iq)
                if sp < 2:
                    for _ in range(2):
                        if filler:
                            name, hp = filler.pop(0)
                            emit_qk_chunk(name, hp, 1, use_act=False)
                else:
                    if sp == 2 and iq == 0:
                        aps_ctx.close()
                        p_Cps = ctx.enter_context(
                            tc.tile_pool(name="C_ps", bufs=1, space="PSUM")
                        )
                        emit_norm(0)
                    elif sp == 2 and iq == 1:
                        emit_norm(1)
                    elif sp == 2 and iq == 2:
                        emit_proj(0)
                    elif sp == 2 and iq == 3:
                        emit_proj(1)
                    elif sp == 3 and iq == 0:
                        emit_norm(2)
                        emit_proj(2)
                    elif sp == 3 and iq == 1:
                        emit_proj(3)
                    elif sp == 3 and iq == 2:
                        emit_proj(4)
                    elif sp == 3 and iq == 3:
                        emit_proj(5)
        emit_norm(3)
        for tt in (6, 7):
            emit_proj(tt)

    nc.compile()
    return nc


def make_tri():
    x = np.arange(128, dtype=np.int32)[:, None]
    y = np.arange(128, dtype=np.int32)[None, :]
    return np.where(y - x >= 0, 0.0, NEG).astype(np.float32)


def shard_inputs(x, Wq, Wk, Wv, proj_w):
    import ml_dtypes
    BF = ml_dtypes.bfloat16
    wqF = np.ascontiguousarray(
        np.transpose(Wq, (1, 0, 2)).reshape(E, DH)
    ).astype(BF)
    wkF = np.ascontiguousarray(
        np.transpose(Wk, (1, 0, 2)).reshape(E, DH)
    ).astype(BF)
    wvF = np.ascontiguousarray(
        np.transpose(Wv, (1, 0, 2)).reshape(E, DH)
    ).astype(BF)
    pwTf = np.ascontiguousarray(proj_w.T).astype(BF)
    tri = make_tri()
    in_maps = []
    for c in range(NCORES):
        b = c // 2
        t0 = TCORE * (c % 2)
        in_maps.append(
            {
                "xT": np.ascontiguousarray(x[b, t0 : t0 + TCORE, :].T).astype(
                    BF
                ),
                "wq": wqF,
                "wk": wkF,
                "wv": wvF,
                "pwT": pwTf,
                "tri": tri,
            }
        )
    return in_maps


_cached_nc = None


def get_nc():
    global _cached_nc
    if _cached_nc is None:
        _cached_nc = build_nc()
    return _cached_nc


def kernel(x, Wq, Wk, Wv, proj_w, proj_b, _trace=False, _tmpdir=None):
    x = np.asarray(x, dtype=np.float32)
    Wq = np.asarray(Wq, dtype=np.float32)
    Wk = np.asarray(Wk, dtype=np.float32)
    Wv = np.asarray(Wv, dtype=np.float32)
    proj_w = np.asarray(proj_w, dtype=np.float32)
    proj_b = np.asarray(proj_b, dtype=np.float32)

    nc = get_nc()
    in_maps = shard_inputs(x, Wq, Wk, Wv, proj_w)
    res = run_bass_kernel_spmd(nc, in_maps, core_ids=list(range(NCORES)))

    out = np.empty((B, T, E), dtype=np.float32)
    for c in range(NCORES):
        b = c // 2
        t0 = TCORE * (c % 2)
        out[b, t0 : t0 + TCORE] = res.results[c]["y"] + proj_b
    return out


# revision 22
# speedup vs baseline: 1.2238x; 1.2238x over previous
"""Causal self-attention nn module (B=4, T=2048, E=1024, H=16, HS=64) on 8
TRN2 cores — faithful to the reference's raw .view() reshape [b,t,h,hs] ->
[h,b,t,hs].

That reshape makes the attention run over 64 independent "sequences": each
sequence is one 128-timestep block of one batch, with its 16 heads
interleaved into 2048 positions (t2 = tau*16 + h).  Sequence (b, s') covers
x[b, 128*s' : 128*(s'+1), :], and its attention output lands back in rows
[128*s', 128*(s'+1)) of att_cat[b] — so sharding by sequence blocks needs no
cross-core reduction at all.

Sharding: core c handles batch b = c//2, rows t in [1024*(c%2), +1024) — 8
sequences.  Each core computes full rows of the output; host concatenates
and adds proj_b.

Dtypes: phase A/C matmuls and V/P tiles are bf16 (PSUM accumulation fp32).
Qseq/Kseq stay fp32r — their PSUM->SBUF scatter writes are 2-byte-strided
in bf16 and run 2.3x slower on DVE, while fp32r matmul is the same
1 cycle/row as bf16 at N>=256.

Schedule (the real optimization): the PE re-throttles (HAM K=4/8 ==
1.2 GHz) whenever it micro-stalls on the softmax chain, so the emission
order is built to keep the PE stream dependency-free:
  - All input DMAs are issued upfront as single whole-tensor transfers
    (the v DRAM round-trip gather is 1 DMA per sequence).
  - Phase A emits only the q/k projections for sequence pairs 0-1 before
    attention starts; the pairs 2-3 projections are interleaved into the
    attention stream of pairs 0-1 as PE filler (their PSUM->Qseq scatter
    copies split across DVE and the otherwise-idle ACT).
  - Attention groups keep score matmuls one k-tile ahead of the P.T @ V
    accumulation; scores for both packed sequences land in one 2-bank
    PSUM tile so exps are merged-pair ACT instructions.
  - Normalization (ones-block denominators -> ln -> exp(-x) -> one
    in-place DVE multiply) is batched per pair and deferred one pair;
    the projection (phase C) for pair sp is interleaved two pairs later,
    giving the PE independent work at every group boundary.
"""

import numpy as np
from contextlib import ExitStack

import concourse.bass as bass
import concourse.mybir as mybir
import concourse.tile as tile
from concourse import bacc
from concourse.bass_utils import run_bass_kernel_spmd

F32 = mybir.dt.float32
F32R = mybir.dt.float32r
BF16 = mybir.dt.bfloat16
AF = mybir.ActivationFunctionType

B, T, E, H, HS = 4, 2048, 1024, 16, 64
NCORES = 8
TCORE = T * B // NCORES      # 1024 rows per core
DH = H * HS                  # 1024
NEG = -1.0e9
SCALE = HS ** -0.5

# band tile geometry: for diagonal-band tile jj, scores only needed in
# columns [128*jj, 512); matmul N kept >=256 for full-rate fp32r.
BAND_C0 = [0, 128, 256, 256]     # first column the st matmul writes
BAND_A0 = [0, 128, 256, 256]     # first column the @v matmul writes


def build_nc(t_core=TCORE, e=E, e_out=E):
    assert t_core % 512 == 0
    nseq = t_core // 128     # sequences (= tau tiles)
    nsp = nseq // 2          # sequence pairs
    ne = e // 128
    nhp = H // 2             # 8 head pairs
    ntk = 16                 # t2 tiles per sequence (2048/128)
    niq = 4                  # t2 query blocks (2048/512)
    neo = e_out // 512

    nc = bacc.Bacc("TRN2", debug=False, num_devices=1)

    xT_d = nc.dram_tensor("xT", [e, t_core], BF16, kind="ExternalInput")
    wq_d = nc.dram_tensor("wq", [e, DH], BF16, kind="ExternalInput")
    wk_d = nc.dram_tensor("wk", [e, DH], BF16, kind="ExternalInput")
    wv_d = nc.dram_tensor("wv", [e, DH], BF16, kind="ExternalInput")
    pw_d = nc.dram_tensor("pwT", [DH, e_out], BF16, kind="ExternalInput")
    tri_d = nc.dram_tensor("tri", [128, 128], F32, kind="ExternalInput")
    y_d = nc.dram_tensor("y", [t_core, e_out], F32, kind="ExternalOutput")

    with tile.TileContext(nc) as tc, ExitStack() as ctx:
        p_keep = ctx.enter_context(tc.tile_pool(name="keep", bufs=1))
        Qseq = p_keep.tile([128, nsp, 2048], F32R, tag="Qseq")
        Kseq = p_keep.tile([128, nsp, 2048], F32R, tag="Kseq")
        tri_sb = p_keep.tile([128, 128], F32, tag="tri")
        nc.sync.dma_start(out=tri_sb, in_=tri_d.ap())

        # per (pair, tk) stationary [128 t2, 192]: cols 0:64 = V of seq A,
        # 64:128 = ones (replicates the softmax denominator across 64 PSUM
        # partitions in the @v matmul), 128:192 = V of seq B
        p_vsb = ctx.enter_context(tc.tile_pool(name="vsb", bufs=1))
        v_sb = p_vsb.tile([128, nsp, ntk, 192], BF16, tag="v")
        nc.vector.memset(v_sb, 1.0)

        p_drm = ctx.enter_context(tc.tile_pool(name="drm", bufs=1, space="DRAM"))
        vscr = p_drm.tile([t_core, DH], BF16, tag="vscr")

        # pools are stack-scoped per memory space: wv sits on top of the
        # SBUF stack (closed right after the v matmuls), Aps on top of the
        # PSUM stack (closed when the projection pool opens)
        p_x = ctx.enter_context(tc.tile_pool(name="px", bufs=1))
        p_w = ctx.enter_context(tc.tile_pool(name="pw", bufs=1))
        p_stg = ctx.enter_context(tc.tile_pool(name="vstg", bufs=2))
        p_st = ctx.enter_context(tc.tile_pool(name="st_ps", bufs=2, space="PSUM"))
        p_av = ctx.enter_context(tc.tile_pool(name="av_ps", bufs=2, space="PSUM"))
        wv_ctx = ExitStack()
        p_wv = wv_ctx.enter_context(tc.tile_pool(name="pwv", bufs=1))
        aps_ctx = ExitStack()
        p_Aps = aps_ctx.enter_context(
            tc.tile_pool(name="Aps", bufs=2, space="PSUM")
        )

        # ---- input DMAs, one per tensor (x split for earlier start) ----
        xT = p_x.tile([128, ne, t_core], BF16, tag="xT")
        x_r = xT_d.ap().rearrange("(a p) t -> p a t", p=128)
        w_sb = {}

        def load_w(name, dram, pool):
            w = pool.tile([128, ne, DH], BF16, tag=name)
            nc.sync.dma_start(
                out=w, in_=dram.ap().rearrange("(a p) d -> p a d", p=128)
            )
            w_sb[name] = w

        # x and wv first (the v chains need them); wq/wk can land later
        nc.sync.dma_start(out=xT[:, 0 : ne // 2, :], in_=x_r[:, 0 : ne // 2, :])
        load_w("wv", wv_d, p_wv)
        nc.sync.dma_start(out=xT[:, ne // 2 :, :], in_=x_r[:, ne // 2 :, :])
        load_w("wq", wq_d, p_w)
        load_w("wk", wk_d, p_w)

        # ---------------- phase A (v + q/k for pairs 0-1) ----------------
        # v -> DRAM scratch -> gathered V tiles (t2 on partitions); the
        # round-trip latency hides under the q/k matmuls.  The DVE copy
        # also casts fp32 PSUM -> bf16 (DMA cannot convert dtypes).
        for tt in range(nseq):
            stg = p_stg.tile([128, DH], BF16, tag="stg")
            for c2 in range(DH // 512):
                ps = p_Aps.tile([128, 512], F32, tag="Aps", name="Aps")
                for ei in range(ne):
                    nc.tensor.matmul(
                        ps,
                        xT[:, ei, bass.ts(tt, 128)],
                        w_sb["wv"][:, ei, bass.ts(c2, 512)],
                        start=(ei == 0),
                        stop=(ei == ne - 1),
                    )
                nc.vector.tensor_copy(stg[:, bass.ts(c2, 512)], ps)
            nc.sync.dma_start(out=vscr[bass.ts(tt, 128), :], in_=stg)
        wv_ctx.close()
        for j in range(nseq):
            src = vscr[j * 128 : (j + 1) * 128, :].rearrange(
                "(b a) (h c) -> (a h) b c", a=8, c=64
            )
            c0 = 128 * (j % 2)
            nc.sync.dma_start(
                out=v_sb[:, j // 2, :, c0 : c0 + 64], in_=src
            )

        def emit_qk_chunk(name, hp, c, use_act):
            # one PSUM chain of the q/k projection + its scatter into the
            # interleaved-transposed layout: seq j = 4c+j4, head h = 2hp+hh,
            # col t2 = tau*16 + h
            dst = Qseq if name == "wq" else Kseq
            w = w_sb[name]
            ps = p_Aps.tile([128, 512], F32, tag="Aps", name="Aps")
            for ei in range(ne):
                nc.tensor.matmul(
                    ps,
                    w[:, ei, hp * 128 : hp * 128 + 128],
                    xT[:, ei, bass.ts(c, 512)],
                    start=(ei == 0),
                    stop=(ei == ne - 1),
                )
            for hh in range(2):
                h = 2 * hp + hh
                pv = ps[64 * hh : 64 * hh + 64, :].rearrange(
                    "p (j4 tau) -> p j4 tau", j4=4
                )
                for par in range(2):
                    dv = dst[64 * par : 64 * par + 64, :, :].rearrange(
                        "p sp (tau hx) -> p sp tau hx", hx=16
                    )
                    if use_act and hh == 1:
                        nc.scalar.activation(
                            dv[:, 2 * c : 2 * c + 2, :, h],
                            pv[:, par::2, :],
                            AF.Copy,
                        )
                    else:
                        nc.vector.tensor_copy(
                            dv[:, 2 * c : 2 * c + 2, :, h],
                            pv[:, par::2, :],
                        )

        for name in ("wq", "wk"):
            for hp in range(nhp):
                emit_qk_chunk(name, hp, 0, use_act=True)

        # q/k for pairs 2-3: emitted as filler inside the attention stream
        filler = [
            (name, hp) for name in ("wq", "wk") for hp in range(nhp)
        ]

        # ---------------- phase B + interleaved A-tail and C ----------------
        p_big = ctx.enter_context(tc.tile_pool(name="big", bufs=1))
        attC = p_big.tile([128, 8, t_core], BF16, tag="attC")
        pwT = p_big.tile([128, 8, e_out], BF16, tag="pwT")
        nc.sync.dma_start(
            out=pwT, in_=pw_d.ap().rearrange("(g p) E -> p g E", p=128)
        )

        p_at = ctx.enter_context(tc.tile_pool(name="attn", bufs=3))
        p_nrm = ctx.enter_context(tc.tile_pool(name="nrm", bufs=2))
        p_dns = ctx.enter_context(tc.tile_pool(name="dns", bufs=3))
        p_dnl = ctx.enter_context(tc.tile_pool(name="dnl", bufs=1))
        p_out = ctx.enter_context(tc.tile_pool(name="out", bufs=2))
        p_Cps = None  # opened after the phase-A PSUM pool closes

        dens_t = {}

        def emit_norm(sp):
            # 1/denominator = exp(-ln d) batched for the whole pair, then
            # one fused in-place multiply over attC's two column blocks
            dens = dens_t.pop(sp)
            dnl = p_dnl.tile([128, 8, 256], F32, tag="dnl", name="dnl")
            nc.scalar.activation(dnl, dens, AF.Ln)
            nc.scalar.activation(dens, dnl, AF.Exp, scale=-1.0)
            sl_ = attC[:, :, 2 * sp * 128 : 2 * sp * 128 + 256]
            nc.vector.tensor_mul(sl_, sl_, dens)

        def emit_proj(tt):
            for eh in range(neo):
                ps = p_Cps.tile([128, 512], F32, tag="Cps", name="Cps")
                for g in range(8):
                    nc.tensor.matmul(
                        ps,
                        attC[:, g, bass.ts(tt, 128)],
                        pwT[:, g, bass.ts(eh, 512)],
                        start=(g == 0),
                        stop=(g == 7),
                    )
                y_sb = p_out.tile([128, 512], F32, tag="y")
                nc.vector.tensor_copy(y_sb, ps)
                nc.sync.dma_start(
                    out=y_d.ap()[bass.ts(tt, 128), bass.ts(eh, 512)],
                    in_=y_sb,
                )

        def emit_group(sp, iq):
            n_tk = 4 * (iq + 1)
            av = {}
            for hh in range(2):
                av[hh] = p_av.tile([128, 512], F32, tag="av", name="av")
            sts = {}

            def emit_st(tk):
                jj = tk - 4 * iq
                c0 = BAND_C0[jj] if jj >= 0 else 0
                t_ = p_st.tile([128, 2, 512], F32, tag="st", name="st")
                for hh in range(2):
                    sl = slice(64 * hh, 64 * hh + 64)
                    nc.tensor.matmul(
                        t_[:, hh, c0:512],
                        Kseq[sl, sp, bass.ts(tk, 128)],
                        Qseq[sl, sp, iq * 512 + c0 : (iq + 1) * 512],
                        start=True,
                        stop=True,
                    )
                sts[tk] = t_

            emit_st(0)
            for tk in range(n_tk):
                if tk + 1 < n_tk:
                    emit_st(tk + 1)
                jj = tk - 4 * iq
                st = sts.pop(tk)
                pt = p_at.tile([128, 2, 512], BF16, tag="pt", name="pt")
                if jj < 0:
                    nc.scalar.activation(pt, st, AF.Exp, scale=SCALE)
                    a0 = 0
                else:
                    d0 = 128 * jj
                    if d0 > 0:
                        nc.vector.memset(pt[:, :, 0:d0], 0.0)
                    sm = p_nrm.tile([128, 2, 128], F32, tag="sm", name="sm")
                    for hh in range(2):
                        nc.vector.tensor_add(
                            sm[:, hh, :], st[:, hh, d0 : d0 + 128], tri_sb
                        )
                    nc.scalar.activation(
                        pt[:, :, d0 : d0 + 128], sm, AF.Exp, scale=SCALE
                    )
                    if d0 + 128 < 512:
                        nc.scalar.activation(
                            pt[:, :, d0 + 128 : 512],
                            st[:, :, d0 + 128 : 512],
                            AF.Exp,
                            scale=SCALE,
                        )
                    a0 = BAND_A0[jj]
                for hh in range(2):
                    nc.tensor.matmul(
                        av[hh][:, a0:512],
                        v_sb[:, sp, tk, 64 * hh : 64 * hh + 128],
                        pt[:, hh, a0:512],
                        start=(tk == 0),
                        stop=(tk == n_tk - 1),
                        skip_group_check=True,
                    )
            # av rows: hh=0 -> out 0:64, denominator 64:128; hh=1 mirrored.
            # Copy out rows unnormalized into attC.T layout; stash the
            # denominators in matching layout for the per-pair batch.
            dens = dens_t[sp]
            for hh in range(2):
                j = 2 * sp + hh
                o0 = 64 * hh
                d0 = 64 - o0
                avv = av[hh][o0 : o0 + 64, :].rearrange(
                    "p (tau g r) -> p r g tau", tau=32, g=8, r=2
                )
                dvv = av[hh][d0 : d0 + 64, :].rearrange(
                    "p (tau g r) -> p r g tau", tau=32, g=8, r=2
                )
                for r in range(2):
                    nc.vector.tensor_copy(
                        attC[
                            64 * r : 64 * r + 64,
                            :,
                            j * 128 + iq * 32 : j * 128 + iq * 32 + 32,
                        ],
                        avv[:, r, :, :],
                    )
                    nc.vector.tensor_copy(
                        dens[
                            64 * r : 64 * r + 64,
                            :,
                            hh * 128 + iq * 32 : hh * 128 + iq * 32 + 32,
                        ],
                        dvv[:, r, :, :],
                    )

        # interleave schedule: pair 0/1 groups carry the pairs-2-3 q/k
        # filler; pair 2/3 groups carry normalization + projection filler
        for sp in range(nsp):
            dens_t[sp] = p_dns.tile(
                [128, 8, 256], BF16, tag="dens", name="dens"
            )
            for iq in range(niq):
                emit_group(sp, iq)
                if sp < 2:
                    for _ in range(2):
                        if filler:
                            name, hp = filler.pop(0)
                            emit_qk_chunk(name, hp, 1, use_act=False)
                    # norm for a pair is emitted one pair later (its ACT
                    # and DVE work hide under the next pair's stream), so
                    # the projection filler is ready the moment it's needed
                    if sp == 1 and iq == 0:
                        emit_norm(0)
                else:
                    if sp == 2 and iq == 0:
                        aps_ctx.close()
                        p_Cps = ctx.enter_context(
                            tc.tile_pool(name="C_ps", bufs=2, space="PSUM")
                        )
                        emit_proj(0)
                        emit_norm(1)
                    elif sp == 2 and iq == 1:
                        emit_proj(1)
                    elif sp == 2 and iq == 2:
                        emit_proj(2)
                    elif sp == 2 and iq == 3:
                        emit_proj(3)
                    elif sp == 3 and iq == 0:
                        emit_norm(2)
                    elif sp == 3 and iq == 1:
                        emit_proj(4)
                    elif sp == 3 and iq == 2:
                        emit_proj(5)
        emit_norm(3)
        for tt in (6, 7):
            emit_proj(tt)

    nc.compile()
    return nc


def make_tri():
    x = np.arange(128, dtype=np.int32)[:, None]
    y = np.arange(128, dtype=np.int32)[None, :]
    return np.where(y - x >= 0, 0.0, NEG).astype(np.float32)


def shard_inputs(x, Wq, Wk, Wv, proj_w):
    import ml_dtypes
    BF = ml_dtypes.bfloat16
    wqF = np.ascontiguousarray(
        np.transpose(Wq, (1, 0, 2)).reshape(E, DH)
    ).astype(BF)
    wkF = np.ascontiguousarray(
        np.transpose(Wk, (1, 0, 2)).reshape(E, DH)
    ).astype(BF)
    wvF = np.ascontiguousarray(
        np.transpose(Wv, (1, 0, 2)).reshape(E, DH)
    ).astype(BF)
    pwTf = np.ascontiguousarray(proj_w.T).astype(BF)
    tri = make_tri()
    in_maps = []
    for c in range(NCORES):
        b = c // 2
        t0 = TCORE * (c % 2)
        in_maps.append(
            {
                "xT": np.ascontiguousarray(x[b, t0 : t0 + TCORE, :].T).astype(
                    BF
                ),
                "wq": wqF,
                "wk": wkF,
                "wv": wvF,
                "pwT": pwTf,
                "tri": tri,
            }
        )
    return in_maps


_cached_nc = None


def get_nc():
    global _cached_nc
    if _cached_nc is None:
        _cached_nc = build_nc()
    return _cached_nc


def kernel(x, Wq, Wk, Wv, proj_w, proj_b, _trace=False, _tmpdir=None):
    x = np.asarray(x, dtype=np.float32)
    Wq = np.asarray(Wq, dtype=np.float32)
    Wk = np.asarray(Wk, dtype=np.float32)
    Wv = np.asarray(Wv, dtype=np.float32)
    proj_w = np.asarray(proj_w, dtype=np.float32)
    proj_b = np.asarray(proj_b, dtype=np.float32)

    nc = get_nc()
    in_maps = shard_inputs(x, Wq, Wk, Wv, proj_w)
    res = run_bass_kernel_spmd(nc, in_maps, core_ids=list(range(NCORES)))

    out = np.empty((B, T, E), dtype=np.float32)
    for c in range(NCORES):
        b = c // 2
        t0 = TCORE * (c % 2)
        out[b, t0 : t0 + TCORE] = res.results[c]["y"] + proj_b
    return out
